# revision 13
# baseline (speedup 1.0000x reference)
"""DistMult decoder kernel for 8 Trainium2 NeuronCores.

Computes out = (input1 * weight[type_index]) @ input2.T + bias with
input1 [8192, 512], input2 [8192, 512] in fp32, out [8192, 8192].

Sharding: rows of input1 (and thus rows of the output) are split across
the 8 cores; input2 / weight / bias are replicated. No communication.

Per-core device program (M = 1024 rows):
  - lhsT  [MT, P, KT*128] = w_r-scaled shard of input1, packed on host
    into per-m-tile blocks (one contiguous 128 KB DMA per m-tile)
  - rhs   [512, 8192]  = input2 transposed + cast to fp16 on host
  - fp16 operands run the PE at 1 cycle/row with fp32 PSUM accumulation
  - GEMM over 16 n-groups of 512 cols x 8 m-tiles x 4 k matmuls;
    215.8 ns/matmul steady state (the N=512 fp16 streaming floor)
  - output stored as fp16 (16 MB/core instead of 32) and upcast on the
    host; total quantization error ~5e-4 vs the 2e-2 gate

Schedule rationale (from trace analysis): ~6 us fixed preamble; first
DMA data lands ~8.7 us; during the head window all 8 cores load
simultaneously so per-core aggregate is only ~180-300 GB/s. 512-col
n-groups keep the critical first-matmul set small (512 KB rhs + 128 KB
lhsT), spread round-robin over the three DGE rings in consumption
order. Warmup matmuls on zeroed SBUF keep the PE busy from ~7.6 us so
the HAM clock gate opens (2.4 GHz) before the real stream starts.
"""

import os

import numpy as np

import concourse.bacc as bacc
import concourse.mybir as mybir
from concourse.bass_utils import run_bass_kernel_spmd
from concourse.tile import TileContext

N_CORES = 8
N1, N2, D = 8192, 8192, 512
M = N1 // N_CORES  # rows per core
P = 128            # partitions
KT = D // P        # 4 k-tiles
MT = M // P        # 8 m-tiles
NG = 512           # n columns per group (one psum bank)
NT = N2 // NG      # 16 n-groups
NWARM = 11         # warmup matmuls: spans PE-ready (~7.5 us) to data-ready (~12.2)

TRACE = os.environ.get("BASS_KERNEL_TRACE", "0") == "1"
LAST_RESULTS = None

_cached_nc = None


def _build():
    nc = bacc.Bacc(
        "TRN2", target_bir_lowering=False, debug=False, enable_asserts=False, num_devices=N_CORES
    )
    f32 = mybir.dt.float32
    f16 = mybir.dt.float16
    lhsT = nc.dram_tensor("lhsT", [MT, P, KT * P], f16, kind="ExternalInput")
    rhs = nc.dram_tensor("rhs", [D, N2], f16, kind="ExternalInput")
    biasv = nc.dram_tensor("biasv", [P, 1], f32, kind="ExternalInput")
    out = nc.dram_tensor("out", [M, N2], f16, kind="ExternalOutput")

    # K-major DRAM view split into [P, KT, cols]: rhs_r[p, kt, n] is
    # rhs row kt*128+p, matching the per-k-tile partition layout.
    rhs_r = rhs[:, :].rearrange("(kt p) n -> p kt n", p=P)

    with TileContext(nc) as tc:
        with (
            tc.tile_pool(name="const", bufs=1) as constp,
            tc.tile_pool(name="lhs", bufs=1) as lhsp,
            tc.tile_pool(name="rhsp", bufs=4) as rhsp,
            tc.tile_pool(name="outp", bufs=8) as outp,
            tc.tile_pool(name="psum", bufs=4, space="PSUM") as psump,
        ):
            # Warmup tiles zeroed on GpSimd (ready first after preamble).
            warm_w = constp.tile([P, P], f16, tag="warmw")
            warm_r = constp.tile([P, NG], f16, tag="warmr")
            nc.gpsimd.memset(warm_w[:], 0.0)
            nc.gpsimd.memset(warm_r[:], 0.0)

            lt = lhsp.tile([P, MT, KT * P], f16, tag="lhs")
            bias_t = constp.tile([P, 1], f32, tag="bias")
            rts = {}

            def rtile(g):
                rt = rhsp.tile([P, KT, NG], f16, tag="rhs")
                rts[g] = rt
                return rt

            rt0, rt1, rt2 = rtile(0), rtile(1), rtile(2)

            # Priority-ordered head loads. Each piece becomes consumable
            # at roughly 8.7us + cumulative-ring-transfer (~110 GB/s per
            # ring) + a flat ~2.3us completion-semaphore lag, so the
            # three ring-first slots carry rt0 k0/k1 + lt m0, and later
            # pieces are ordered by (and sized to) their deadlines.
            nc.sync.dma_start(out=rt0[:, 0, :], in_=rhs_r[:, 0, 0:NG])
            nc.scalar.dma_start(out=bias_t[:], in_=biasv[:, :])
            nc.scalar.dma_start(out=lt[:, 0, :], in_=lhsT[0, :, :])
            nc.gpsimd.dma_start(out=rt0[:, 1, :], in_=rhs_r[:, 1, 0:NG])
            nc.sync.dma_start(out=rt0[:, 2, :], in_=rhs_r[:, 2, 0:NG])
            nc.scalar.dma_start(out=rt0[:, 3, :], in_=rhs_r[:, 3, 0:NG])
            nc.gpsimd.dma_start(out=lt[:, 1, :], in_=lhsT[1, :, :])
            nc.sync.dma_start(out=lt[:, 2, :], in_=lhsT[2, :, :])
            nc.scalar.dma_start(out=lt[:, 3, :], in_=lhsT[3, :, :])
            nc.gpsimd.dma_start(out=lt[:, 4, :], in_=lhsT[4, :, :])
            nc.sync.dma_start(
                out=rt1[:, 0:2, :], in_=rhs_r[:, 0:2, NG : 2 * NG]
            )
            nc.scalar.dma_start(
                out=rt1[:, 2:4, :], in_=rhs_r[:, 2:4, NG : 2 * NG]
            )
            nc.sync.dma_start(out=lt[:, 5, :], in_=lhsT[5, :, :])
            nc.scalar.dma_start(out=lt[:, 6, :], in_=lhsT[6, :, :])
            nc.gpsimd.dma_start(out=lt[:, 7, :], in_=lhsT[7, :, :])
            nc.gpsimd.dma_start(
                out=rt2[:], in_=rhs_r[:, :, 2 * NG : 3 * NG]
            )

            # Warm up the PE's HAM clock gate during the head-load
            # window so the real matmuls start at 2.4 GHz.
            wps = psump.tile([P, NG], f32, tag="ps")
            for i in range(NWARM):
                nc.tensor.matmul(
                    wps[:], warm_w[:], warm_r[:],
                    start=(i == 0), stop=(i == NWARM - 1),
                )

            # Steady-state rhs prefetch on the GpSimd (SWDGE) queue:
            # latency-tolerant, never behind the HWDGE store streams.
            def load_rhs(g):
                rt = rtile(g)
                nc.gpsimd.dma_start(
                    out=rt[:], in_=rhs_r[:, :, g * NG : (g + 1) * NG]
                )

            for g in range(NT):
                rt = rts.pop(g)
                for m in range(MT):
                    if m == 0 and 3 <= g + 2 < NT:
                        load_rhs(g + 2)
                    last = g == NT - 1 and m == MT - 1
                    ps = psump.tile([P, NG], f32, tag="ps")
                    for k in range(KT):
                        nc.tensor.matmul(
                            ps[:], lt[:, m, k * P : (k + 1) * P],
                            rt[:, k, :],
                            start=(k == 0), stop=(k == KT - 1),
                        )
                    ot = outp.tile([P, NG], f16, tag="ot")
                    if last:
                        # Final tile: split the copy between ACT and DVE
                        # and the store over both HWDGE rings so the
                        # exit barrier waits on minimal serial work.
                        nc.scalar.activation(
                            ot[:, 0:256], ps[:, 0:256],
                            mybir.ActivationFunctionType.Identity,
                            bias=bias_t[:, 0:1],
                        )
                        nc.vector.tensor_scalar_add(
                            ot[:, 256:NG], ps[:, 256:NG], bias_t[:, 0:1]
                        )
                        nc.sync.dma_start(
                            out=out[m * P : (m + 1) * P,
                                    g * NG : g * NG + 256],
                            in_=ot[:, 0:256],
                        )
                        nc.scalar.dma_start(
                            out=out[m * P : (m + 1) * P,
                                    g * NG + 256 : (g + 1) * NG],
                            in_=ot[:, 256:NG],
                        )
                    else:
                        # Alternate psum->sbuf+bias copies between ACT
                        # and DVE, and stores between the HWDGE rings.
                        if m % 2 == 0:
                            nc.scalar.activation(
                                ot[:], ps[:],
                                mybir.ActivationFunctionType.Identity,
                                bias=bias_t[:, 0:1],
                            )
                        else:
                            nc.vector.tensor_scalar_add(
                                ot[:], ps[:], bias_t[:, 0:1]
                            )
                        st = nc.sync if m % 2 == 0 else nc.scalar
                        st.dma_start(
                            out=out[m * P : (m + 1) * P,
                                    g * NG : (g + 1) * NG],
                            in_=ot[:],
                        )
    nc.compile()
    return nc


def kernel(input1, input2, weight, bias, type_index):
    global _cached_nc, LAST_RESULTS

    input1 = np.asarray(input1, dtype=np.float32)
    input2 = np.asarray(input2, dtype=np.float32)
    weight = np.asarray(weight, dtype=np.float32)
    bias = np.asarray(bias, dtype=np.float32).reshape(-1)
    w_r = weight[int(type_index)]  # [D]

    # Host-side prep: fold the w_r row-scale into input1, lay both GEMM
    # operands out K-major, cast to fp16 (device accumulates in fp32).
    scaled = input1 * w_r[None, :]  # [N1, D]
    rhsT = np.ascontiguousarray(input2.T.astype(np.float16))  # [D, N2]
    bias_vec = np.full((P, 1), float(bias[0]), dtype=np.float32)

    in_maps = []
    for c in range(N_CORES):
        shard = scaled[c * M : (c + 1) * M]  # [M, D]
        # Pack per-m-tile weight blocks: lhsT[m, p, k*128+j] =
        # shard[m*128+j, k*128+p], so each m-tile is one contiguous DMA
        # and each k slice is a [K=128, M=128] stationary operand.
        a = shard.T.astype(np.float16).reshape(KT, P, MT, P)
        lhsT_packed = np.ascontiguousarray(
            a.transpose(2, 1, 0, 3).reshape(MT, P, KT * P)
        )
        in_maps.append(
            {
                "lhsT": lhsT_packed,
                "rhs": rhsT,
                "biasv": bias_vec,
            }
        )

    if _cached_nc is None:
        _cached_nc = _build()

    res = run_bass_kernel_spmd(
        _cached_nc, in_maps, core_ids=list(range(N_CORES)), trace=TRACE
    )
    LAST_RESULTS = res
    return np.concatenate(
        [res.results[c]["out"] for c in range(N_CORES)], axis=0
    ).astype(np.float32)


# revision 15
# speedup vs baseline: 1.0137x; 1.0137x over previous
"""DistMult decoder kernel for 8 Trainium2 NeuronCores.

Computes out = (input1 * weight[type_index]) @ input2.T + bias with
input1 [8192, 512], input2 [8192, 512] in fp32, out [8192, 8192].

Sharding: rows of input1 (and thus rows of the output) are split across
the 8 cores; input2 / weight / bias are replicated. No communication.

Per-core device program (M = 1024 rows):
  - lhsT  [MT, P, KT*128] = w_r-scaled shard of input1, packed on host
    into per-m-tile blocks (one contiguous 128 KB DMA per m-tile)
  - rhs   [512, 8192]  = input2 transposed + cast to fp16 on host
  - fp16 operands run the PE at 1 cycle/row with fp32 PSUM accumulation
  - GEMM over 16 n-groups of 512 cols x 8 m-tiles x 4 k matmuls;
    215.8 ns/matmul steady state (the N=512 fp16 streaming floor)
  - output stored as fp16 (16 MB/core instead of 32) and upcast on the
    host; total quantization error ~5e-4 vs the 2e-2 gate

Schedule rationale (from trace analysis): ~6 us fixed preamble; first
DMA data lands ~8.7 us; during the head window all 8 cores load
simultaneously so per-core aggregate is only ~180-300 GB/s. 512-col
n-groups keep the critical first-matmul set small (512 KB rhs + 128 KB
lhsT), spread round-robin over the three DGE rings in consumption
order. Warmup matmuls on zeroed SBUF keep the PE busy from ~7.6 us so
the HAM clock gate opens (2.4 GHz) before the real stream starts.
"""

import os

import numpy as np

import concourse.bacc as bacc
import concourse.mybir as mybir
from concourse.bass_utils import run_bass_kernel_spmd
from concourse.tile import TileContext

N_CORES = 8
N1, N2, D = 8192, 8192, 512
M = N1 // N_CORES  # rows per core
P = 128            # partitions
KT = D // P        # 4 k-tiles
MT = M // P        # 8 m-tiles
NG = 512           # n columns per group (one psum bank)
NT = N2 // NG      # 16 n-groups
NWARM = 12         # warmup matmuls: spans PE-ready (~7.6 us) to data-ready (~12.9)

TRACE = os.environ.get("BASS_KERNEL_TRACE", "0") == "1"
LAST_RESULTS = None

_cached_nc = None


def _build():
    nc = bacc.Bacc(
        "TRN2", target_bir_lowering=False, debug=False, enable_asserts=False, num_devices=N_CORES
    )
    f32 = mybir.dt.float32
    f16 = mybir.dt.float16
    lhsT = nc.dram_tensor("lhsT", [MT, P, KT * P], f16, kind="ExternalInput")
    rhs = nc.dram_tensor("rhs", [D, N2], f16, kind="ExternalInput")
    biasv = nc.dram_tensor("biasv", [P, 1], f32, kind="ExternalInput")
    out = nc.dram_tensor("out", [M, N2], f16, kind="ExternalOutput")

    # K-major DRAM view split into [P, KT, cols]: rhs_r[p, kt, n] is
    # rhs row kt*128+p, matching the per-k-tile partition layout.
    rhs_r = rhs[:, :].rearrange("(kt p) n -> p kt n", p=P)

    with TileContext(nc) as tc:
        with (
            tc.tile_pool(name="const", bufs=1) as constp,
            tc.tile_pool(name="lhs", bufs=1) as lhsp,
            tc.tile_pool(name="rhsp", bufs=4) as rhsp,
            tc.tile_pool(name="outp", bufs=8) as outp,
            tc.tile_pool(name="psum", bufs=4, space="PSUM") as psump,
        ):
            # Warmup tiles zeroed on GpSimd (ready first after preamble).
            warm_w = constp.tile([P, P], f16, tag="warmw")
            warm_r = constp.tile([P, NG], f16, tag="warmr")
            nc.gpsimd.memset(warm_w[:], 0.0)
            nc.gpsimd.memset(warm_r[:], 0.0)

            lt = lhsp.tile([P, MT, KT * P], f16, tag="lhs")
            bias_t = constp.tile([P, 1], f32, tag="bias")
            rts = {}

            def rtile(g):
                rt = rhsp.tile([P, KT, NG], f16, tag="rhs")
                rts[g] = rt
                return rt

            rt0, rt1 = rtile(0), rtile(1)

            # Priority-ordered head loads, round-robin across the three
            # DGE rings in consumption order (~128 KB pieces). The real
            # stream needs rt0 + lt m0 first; g1's quarters and the
            # later m-tiles interleave by their deadlines.
            nc.scalar.dma_start(out=bias_t[:], in_=biasv[:, :])
            nc.sync.dma_start(out=rt0[:, 0, :], in_=rhs_r[:, 0, 0:NG])
            nc.scalar.dma_start(out=lt[:, 0, :], in_=lhsT[0, :, :])
            nc.gpsimd.dma_start(out=rt0[:, 1, :], in_=rhs_r[:, 1, 0:NG])
            nc.sync.dma_start(out=rt0[:, 2, :], in_=rhs_r[:, 2, 0:NG])
            nc.scalar.dma_start(out=rt0[:, 3, :], in_=rhs_r[:, 3, 0:NG])
            nc.gpsimd.dma_start(out=lt[:, 1, :], in_=lhsT[1, :, :])
            nc.sync.dma_start(out=lt[:, 2, :], in_=lhsT[2, :, :])
            nc.scalar.dma_start(out=lt[:, 3, :], in_=lhsT[3, :, :])
            nc.gpsimd.dma_start(out=lt[:, 4, :], in_=lhsT[4, :, :])
            nc.sync.dma_start(out=rt1[:, 0, :], in_=rhs_r[:, 0, NG : 2 * NG])
            nc.scalar.dma_start(out=lt[:, 5, :], in_=lhsT[5, :, :])
            nc.gpsimd.dma_start(out=rt1[:, 1, :], in_=rhs_r[:, 1, NG : 2 * NG])
            nc.sync.dma_start(out=lt[:, 6, :], in_=lhsT[6, :, :])
            nc.scalar.dma_start(out=rt1[:, 2, :], in_=rhs_r[:, 2, NG : 2 * NG])
            nc.gpsimd.dma_start(out=lt[:, 7, :], in_=lhsT[7, :, :])
            nc.sync.dma_start(out=rt1[:, 3, :], in_=rhs_r[:, 3, NG : 2 * NG])

            # Warm up the PE's HAM clock gate during the head-load
            # window so the real matmuls start at 2.4 GHz.
            wps = psump.tile([P, NG], f32, tag="ps")
            for i in range(NWARM):
                nc.tensor.matmul(
                    wps[:], warm_w[:], warm_r[:],
                    start=(i == 0), stop=(i == NWARM - 1),
                )

            # Steady-state rhs prefetch on the GpSimd (SWDGE) queue:
            # latency-tolerant, never behind the HWDGE store streams.
            def load_rhs(g):
                rt = rtile(g)
                nc.gpsimd.dma_start(
                    out=rt[:], in_=rhs_r[:, :, g * NG : (g + 1) * NG]
                )

            for g in range(NT):
                rt = rts.pop(g)
                for m in range(MT):
                    if m == 0 and 2 <= g + 2 < NT:
                        load_rhs(g + 2)
                    last = g == NT - 1 and m == MT - 1
                    ps = psump.tile([P, NG], f32, tag="ps")
                    for k in range(KT):
                        nc.tensor.matmul(
                            ps[:], lt[:, m, k * P : (k + 1) * P],
                            rt[:, k, :],
                            start=(k == 0), stop=(k == KT - 1),
                        )
                    ot = outp.tile([P, NG], f16, tag="ot")
                    if last:
                        # Final tile: split the copy between ACT and DVE
                        # and the store over both HWDGE rings so the
                        # exit barrier waits on minimal serial work.
                        nc.scalar.activation(
                            ot[:, 0:256], ps[:, 0:256],
                            mybir.ActivationFunctionType.Identity,
                            bias=bias_t[:, 0:1],
                        )
                        nc.vector.tensor_scalar_add(
                            ot[:, 256:NG], ps[:, 256:NG], bias_t[:, 0:1]
                        )
                        nc.sync.dma_start(
                            out=out[m * P : (m + 1) * P,
                                    g * NG : g * NG + 256],
                            in_=ot[:, 0:256],
                        )
                        nc.scalar.dma_start(
                            out=out[m * P : (m + 1) * P,
                                    g * NG + 256 : (g + 1) * NG],
                            in_=ot[:, 256:NG],
                        )
                    else:
                        # Alternate psum->sbuf+bias copies between ACT
                        # and DVE, and stores between the HWDGE rings.
                        if m % 2 == 0:
                            nc.scalar.activation(
                                ot[:], ps[:],
                                mybir.ActivationFunctionType.Identity,
                                bias=bias_t[:, 0:1],
                            )
                        else:
                            nc.vector.tensor_scalar_add(
                                ot[:], ps[:], bias_t[:, 0:1]
                            )
                        st = nc.sync if m % 2 == 0 else nc.scalar
                        st.dma_start(
                            out=out[m * P : (m + 1) * P,
                                    g * NG : (g + 1) * NG],
                            in_=ot[:],
                        )
    nc.compile()
    return nc


def kernel(input1, input2, weight, bias, type_index):
    global _cached_nc, LAST_RESULTS

    input1 = np.asarray(input1, dtype=np.float32)
    input2 = np.asarray(input2, dtype=np.float32)
    weight = np.asarray(weight, dtype=np.float32)
    bias = np.asarray(bias, dtype=np.float32).reshape(-1)
    w_r = weight[int(type_index)]  # [D]

    # Host-side prep: fold the w_r row-scale into input1, lay both GEMM
    # operands out K-major, cast to fp16 (device accumulates in fp32).
    scaled = input1 * w_r[None, :]  # [N1, D]
    rhsT = np.ascontiguousarray(input2.T.astype(np.float16))  # [D, N2]
    bias_vec = np.full((P, 1), float(bias[0]), dtype=np.float32)

    in_maps = []
    for c in range(N_CORES):
        shard = scaled[c * M : (c + 1) * M]  # [M, D]
        # Pack per-m-tile weight blocks: lhsT[m, p, k*128+j] =
        # shard[m*128+j, k*128+p], so each m-tile is one contiguous DMA
        # and each k slice is a [K=128, M=128] stationary operand.
        a = shard.T.astype(np.float16).reshape(KT, P, MT, P)
        lhsT_packed = np.ascontiguousarray(
            a.transpose(2, 1, 0, 3).reshape(MT, P, KT * P)
        )
        in_maps.append(
            {
                "lhsT": lhsT_packed,
                "rhs": rhsT,
                "biasv": bias_vec,
            }
        )

    if _cached_nc is None:
        _cached_nc = _build()

    res = run_bass_kernel_spmd(
        _cached_nc, in_maps, core_ids=list(range(N_CORES)), trace=TRACE
    )
    LAST_RESULTS = res
    return np.concatenate(
        [res.results[c]["out"] for c in range(N_CORES)], axis=0
    ).astype(np.float32)
